# revision 15
# baseline (speedup 1.0000x reference)
"""Causal self-attention (B=4, T=2048, C=1024, 16 heads) on 8 TRN2 NeuronCores.

Sharding: core = hg*4 + b  (b = batch 0..3, hg = head-group 0..1, 8 heads each).
Each core computes, for its (batch, head-group):
    qkv^T projection (transposed layout), causal attention in S^T layout
    (scores kept transposed so no P-transposes are needed; softmax denominator
    via an appended ones-column in the V matmul), and the output projection
    against its 512 rows of w_proj.  Host sums the two head-group partials per
    batch and adds b_proj.

The host pre-shards and pre-formats inputs: x arrives transposed ([C, T]) and
in bf16, weights in bf16 — so the device does no casts/transposes and every
DMA is a plain load into a persistent SBUF tile (walrus limits DMA
descriptors to 2 semaphore waits; staging-pool rotation exceeds that).
"""

import sys

if "/opt/trn_rl_repo" not in sys.path:
    sys.path.insert(0, "/opt/trn_rl_repo")

import numpy as np

from concourse import bacc, bass, mybir, tile

F32 = mybir.dt.float32
BF16 = mybir.dt.bfloat16
AF = mybir.ActivationFunctionType
ALU = mybir.AluOpType


def build_nc(T=2048, C=1024, HL=8, D=64, TQB=512):
    """Build the per-core Bass program (all 8 cores run this same NEFF)."""
    P = 128
    CT = C // P              # contraction tiles for the qkv projection
    HP = HL // 2             # head pairs
    FQK = HL * D             # q (or k) feature count for this core
    NFT = FQK // P           # f-tiles for q (same for k); == HP
    TT = T // P              # token tiles
    R = TQB // P             # tk tiles per q-block
    NQ = T // TQB            # q blocks
    VA = HL * 65             # v augmented with a ones column per head
    KO = (HL * D) // P       # contraction tiles for the projection
    NB = min(512, C)         # projection output block
    NO = C // NB

    nc = bacc.Bacc("TRN2", target_bir_lowering=False, debug=False)

    xT_d = nc.dram_tensor("xT", [C, T], BF16, kind="ExternalInput")
    wqkv_d = nc.dram_tensor("w_qkv", [C, 3 * FQK], BF16, kind="ExternalInput")
    bqkv_d = nc.dram_tensor("b_qkv", [3 * FQK], F32, kind="ExternalInput")
    wproj_d = nc.dram_tensor("w_proj", [HL * D, C], BF16, kind="ExternalInput")
    out_d = nc.dram_tensor("out", [T, C], F32, kind="ExternalOutput")

    with tile.TileContext(nc) as tc:
        with (
            tc.tile_pool(name="persist", bufs=1) as pp,
            tc.tile_pool(name="psum", bufs=1, space="PSUM") as psp,
            tc.tile_pool(name="work", bufs=1) as wp,
        ):
            qkT = pp.tile([P, 2 * NFT, T], BF16)      # q^T tiles then k^T tiles
            v_aug = pp.tile([P, TT, VA], BF16)
            oT = pp.tile([P, KO, T], BF16)
            wproj_bf = pp.tile([P, KO, C], BF16)
            bqk_sb = pp.tile([P, 2 * NFT, 1], F32)
            bv_sb = pp.tile([P, HL, 1], F32)          # per-head, at partitions 0..63
            ones_sb = pp.tile([P, 64], BF16)

            nc.gpsimd.memset(ones_sb[:, :], 1.0)
            # ones columns of v_aug (col 64 of each head's 65-wide group)
            va_view = v_aug.rearrange("p t (h e) -> p t h e", e=65)
            nc.gpsimd.memset(va_view[:, :, :, 64:65], 1.0)

            # biases: q/k per-feature (partition dim in qkv^T layout)
            for fi in range(2 * NFT):
                nc.gpsimd.dma_start(
                    bqk_sb[:, fi, :],
                    bqkv_d[fi * P : (fi + 1) * P].rearrange("(p o) -> p o", o=1),
                )
            for hh in range(HL):
                off = 2 * FQK + hh * 64
                nc.gpsimd.dma_start(
                    bv_sb[0:64, hh, :],
                    bqkv_d[off : off + 64].rearrange("(p o) -> p o", o=1),
                )

            for ki in range(KO):
                nc.gpsimd.dma_start(
                    wproj_bf[:, ki, :], wproj_d[ki * P : (ki + 1) * P, :]
                )

            with tc.tile_pool(name="early", bufs=1) as ep:
                xT = ep.tile([P, CT, T], BF16)
                wqkv_bf = ep.tile([P, CT, 3 * FQK], BF16)

                for ci in range(CT):
                    nc.gpsimd.dma_start(
                        xT[:, ci, :], xT_d[ci * P : (ci + 1) * P, :]
                    )
                    nc.gpsimd.dma_start(
                        wqkv_bf[:, ci, :], wqkv_d[ci * P : (ci + 1) * P, :]
                    )

                # ---- q^T, k^T projection (transposed layout) -------------
                TB = min(512, T)
                for fi in range(2 * NFT):
                    wcol = fi * P if fi < NFT else FQK + (fi - NFT) * P
                    for qb in range(T // TB):
                        ps = psp.tile([P, TB], F32, tag="big", bufs=2)
                        for ci in range(CT):
                            nc.tensor.matmul(
                                ps[:, :],
                                wqkv_bf[:, ci, wcol : wcol + P],
                                xT[:, ci, qb * TB : (qb + 1) * TB],
                                start=(ci == 0),
                                stop=(ci == CT - 1),
                            )
                        nc.scalar.activation(
                            qkT[:, fi, qb * TB : (qb + 1) * TB],
                            ps[:, :],
                            AF.Identity,
                            bias=bqk_sb[:, fi, :],
                        )

                # ---- v in natural layout, scattered into v_aug -----------
                for ti in range(TT):
                    ps = psp.tile([P, FQK], F32, tag="big", bufs=2)
                    for ci in range(CT):
                        nc.tensor.matmul(
                            ps[:, :],
                            xT[:, ci, ti * P : (ti + 1) * P],
                            wqkv_bf[:, ci, 2 * FQK : 3 * FQK],
                            start=(ci == 0),
                            stop=(ci == CT - 1),
                        )
                    nc.vector.tensor_copy(
                        va_view[:, ti, :, 0:64],
                        ps.rearrange("p (h e) -> p h e", e=64)[:, :, :],
                    )

            # ---- attention ----------------------------------------------
            for hp in range(HP):
                for qb in range(NQ):
                    n_tk = R * (qb + 1)
                    q0 = qb * TQB
                    pt = {}  # (h, pair) -> [P, 2, TQB] bf16
                    for pi in range(n_tk // 2):
                        for h in (0, 1):
                            sg = psp.tile(
                                [P, 2 * TQB], F32, tag="big", bufs=2
                            )
                            for s in (0, 1):
                                kj = 2 * pi + s
                                nc.tensor.matmul(
                                    sg[:, s * TQB : (s + 1) * TQB],
                                    qkT[
                                        64 * h : 64 * h + 64,
                                        NFT + hp,
                                        kj * P : (kj + 1) * P,
                                    ],
                                    qkT[64 * h : 64 * h + 64, hp, q0 : q0 + TQB],
                                    start=True,
                                    stop=True,
                                )
                            ptt = wp.tile(
                                [P, 2, TQB], BF16, tag=f"pt{h}_{pi}", bufs=1
                            )
                            pt[(h, pi)] = ptt
                            nc.scalar.activation(
                                ptt[:, :, :],
                                sg.rearrange("p (s n) -> p s n", n=TQB)[:, :, :],
                                AF.Exp,
                                scale=float(D) ** -0.5,
                            )
                            for s in (0, 1):
                                j = 2 * pi + s - R * qb
                                if j >= 0:
                                    c0 = 128 * j
                                    nc.gpsimd.affine_select(
                                        ptt[:, s, c0 : c0 + 128],
                                        ptt[:, s, c0 : c0 + 128],
                                        pattern=[[1, 128]],
                                        channel_multiplier=-1,
                                        base=0,
                                        compare_op=ALU.is_ge,
                                        fill=0.0,
                                    )
                    for h in (0, 1):
                        hh = 2 * hp + h
                        po = psp.tile([P, TQB], F32, tag="po", bufs=2)
                        for kj in range(n_tk):
                            j = kj - R * qb
                            c0 = 128 * j if j > 0 else 0
                            nc.tensor.matmul(
                                po[0:65, c0:TQB],
                                va_view[:, kj, hh, :],
                                pt[(h, kj // 2)][:, kj % 2, c0:TQB],
                                start=(kj == 0),
                                stop=(kj == n_tk - 1),
                            )
                        # normalize rows 0..63 by row 64, add v-bias
                        r_sb = wp.tile([P, TQB], BF16, tag="r", bufs=2)
                        with nc.allow_low_precision(reason="softmax denom recip"):
                            nc.vector.reciprocal(r_sb[64:65, :], po[64:65, :])
                        pb = psp.tile([P, TQB], F32, tag="pb", bufs=2)
                        nc.tensor.matmul(
                            pb[0:64, :],
                            ones_sb[64:65, :],
                            r_sb[64:65, :],
                            start=True,
                            stop=True,
                        )
                        sb_b = wp.tile([P, TQB], F32, tag="sbb", bufs=2)
                        nc.scalar.copy(sb_b[0:64, :], pb[0:64, :])
                        if h == 0:
                            ot = oT[0:64, hp, q0 : q0 + TQB]
                            nc.vector.tensor_mul(ot, po[0:64, :], sb_b[0:64, :])
                            nc.scalar.activation(
                                ot, ot, AF.Identity, bias=bv_sb[0:64, hh, :]
                            )
                        else:
                            otmp = wp.tile([P, TQB], BF16, tag="otmp", bufs=2)
                            nc.vector.tensor_mul(
                                otmp[0:64, :], po[0:64, :], sb_b[0:64, :]
                            )
                            nc.scalar.activation(
                                otmp[0:64, :],
                                otmp[0:64, :],
                                AF.Identity,
                                bias=bv_sb[0:64, hh, :],
                            )
                            nc.gpsimd.dma_start(
                                oT[64:128, hp, q0 : q0 + TQB], otmp[0:64, :]
                            )

            # ---- output projection --------------------------------------
            for ti in range(TT):
                for n in range(NO):
                    ps = psp.tile([P, NB], F32, tag="big", bufs=2)
                    for ki in range(KO):
                        nc.tensor.matmul(
                            ps[:, :],
                            oT[:, ki, ti * P : (ti + 1) * P],
                            wproj_bf[:, ki, n * NB : (n + 1) * NB],
                            start=(ki == 0),
                            stop=(ki == KO - 1),
                        )
                    ys = wp.tile([P, NB], F32, tag="y", bufs=3)
                    nc.vector.tensor_copy(ys[:, :], ps[:, :])
                    nc.gpsimd.dma_start(
                        out_d[ti * P : (ti + 1) * P, n * NB : (n + 1) * NB], ys[:, :]
                    )

    nc.compile()
    return nc


def shard_inputs(x, w_attn, b_attn, w_proj, HL=8):
    """Per-core input dicts for core = hg*4 + b.  Host pre-formats: x is
    transposed to [C, T] bf16, weights cast to bf16."""
    import ml_dtypes

    bf = ml_dtypes.bfloat16
    B = x.shape[0]
    C = x.shape[2]
    S = HL * 64                # feature slab per head-group
    n_hg = C // S
    in_maps = []
    for core in range(B * n_hg):
        b, hg = core % B, core // B
        sl = slice(hg * S, (hg + 1) * S)
        w_qkv = np.concatenate(
            [w_attn[:, sl], w_attn[:, C:][:, sl], w_attn[:, 2 * C :][:, sl]], axis=1
        )
        b_qkv = np.concatenate([b_attn[sl], b_attn[C:][sl], b_attn[2 * C :][sl]])
        in_maps.append(
            {
                "xT": np.ascontiguousarray(x[b].T).astype(bf),
                "w_qkv": np.ascontiguousarray(w_qkv).astype(bf),
                "b_qkv": np.ascontiguousarray(b_qkv, dtype=np.float32),
                "w_proj": np.ascontiguousarray(w_proj[sl]).astype(bf),
            }
        )
    return in_maps


_NC_CACHE = {}


def kernel(x, w_attn, b_attn, w_proj, b_proj, trace=False):
    from concourse.bass_utils import run_bass_kernel_spmd

    x = np.asarray(x, dtype=np.float32)
    w_attn = np.asarray(w_attn, dtype=np.float32)
    b_attn = np.asarray(b_attn, dtype=np.float32)
    w_proj = np.asarray(w_proj, dtype=np.float32)
    b_proj = np.asarray(b_proj, dtype=np.float32)

    B, T, C = x.shape
    if "nc" not in _NC_CACHE:
        _NC_CACHE["nc"] = build_nc(T=T, C=C, HL=8, D=64, TQB=512)
    nc = _NC_CACHE["nc"]

    in_maps = shard_inputs(x, w_attn, b_attn, w_proj, HL=8)
    res = run_bass_kernel_spmd(
        nc, in_maps, core_ids=list(range(8)), trace=trace
    )
    y = np.zeros((B, T, C), dtype=np.float32)
    for core in range(8):
        b = core % B
        y[b] += res.results[core]["out"]
    y += b_proj[None, None, :]
    if trace:
        kernel.last_results = res
    return y
